# revision 17
# baseline (speedup 1.0000x reference)
"""BFP (block floating point) activation quantization kernel for Trainium2.

Problem: x [32, 256, 56, 56] f32; per (batch, 32-channel block, h, w) the 32
channels share an exponent e = floor(log2(max |x|)); quantize each value to
q * 2^(e-2) with q = clip(round(x / 2^(e-2)), -7, 7)  (mantissa=3 bits).

Strategy (pure data-parallel over batch, 4 images per core on 8 cores):
  - The host permutes each core's shard into the exact SBUF layout
    [chunk 8][p 128][ci 32][j 98] (p = img*32 + blk*4 + hwq), so every DMA
    is a fully linear 1.6MB transfer with 12.5KB-contiguous descriptor runs
    (the in-place layout's 784B runs measured 62ns/descriptor on the SDMA
    engines, ~144 GB/s; linear runs restore ~408 GB/s).  The output is
    stored bf16 in the same permuted layout and un-permuted/upconverted on
    the host (quantized values carry at most 4 significant bits, so bf16
    is exact).
  - Per chunk: |x| on ScalarE (bf16 out; only the exponent of the maxabs
    survives), maxabs tree as bf16 max levels on DVE 2x (flat APs; the
    last level emits fp32 maxabs directly), exponent bit-ops on [128,98]
    int32, then ONE custom DVE op fuses scale+clip+round:
        r = min(max(x*rscale, -c), c) + MAGIC,  c = nextbefore(7.5)
    (clip-before-round at c is exactly clip-after-round at +-7),
    -MAGIC on ScalarE (bf16 out), out = q*scale in bf16 on GpSimd (DVE for
    the last chunk, which ends the kernel's critical tail).
  - Emission is software-pipelined: chunk k+1's |x| is enqueued on ScalarE
    before chunk k's -MAGIC so the ScalarE queue never blocks the next
    chunk's tree.  Loads issue on the SP HWDGE ring with 2-chunk lookahead;
    stores issue via GpSimd SWDGE right after the GpSimd multiply that
    produces them, so no engine queue ever waits on a foreign producer.
"""

import numpy as np

import concourse.bass as bass
import concourse.tile as tile
from concourse import bacc, mybir
from concourse import dve_ops as _DO
from concourse.bass_utils import run_bass_kernel_spmd
from concourse.dve_spec import C0, C1, Spec, Src0, Src1, lower, maxx, minn
from concourse.dve_uop import DveOpSpec

F32 = mybir.dt.float32
BF16 = mybir.dt.bfloat16
I32 = mybir.dt.int32

N_CORES = 8
B, C, H, W = 32, 256, 56, 56
HW = H * W            # 3136
BPC = B // N_CORES    # 4 images per core
NCHUNK = 8
J = HW // 4 // NCHUNK  # 98 hw-inner elements per chunk
CI = 32               # channels per block
FREE = CI * J         # free elems per chunk per partition
MAGIC = 12582912.0    # 1.5 * 2**23: RNE round-to-integer magic for |v| < 2**22
CLIP_C = 7.499999523162842  # nextbefore(7.5): round(clip(v)) == clip(round(v))

_CACHE = {}


def _register_bfp_op():
    """Custom DVE op: out = min(max(in0*in1, -s1), s1) + s0 (4 ALU stages).

    Fuses the rscale multiply, the +-qmax clip, and the +MAGIC round-add
    into one DVE pass.  in1 is the [128,1,J] rscale broadcast (STT shape).
    """
    name = "BFP_SCALE_CLIP_ROUND"
    for op in _DO.OPS:
        if op.name == name:
            return op
    spec = Spec(
        body=minn(maxx(Src0 * Src1, -C1), C1) + C0,
        reference=lambda in0, in1, s0, s1, imm2: (
            np.minimum(np.maximum(in0 * in1, -s1), s1) + s0
        ).astype(np.float32),
    )
    row = _DO._CUSTOM_DVE_ROW_BASE + len(_DO.OPS)
    shas = {
        ver: DveOpSpec(
            name=name, opcode=row, uops=lower(spec, ver=ver), rd1_en=True
        ).sha(ver)
        for ver in ("v3", "v4")
    }
    op = _DO.DveOp(name, spec, subdim=False, uops_sha=shas)
    _DO.OPS.append(op)
    _DO.CUSTOM_DVE_SPECS[name] = spec
    _DO._SUB_OPCODE_FOR_NAME[name] = row
    return op


_BFP_OP = _register_bfp_op()


def _flat(ap):
    return ap.rearrange("p a b -> p (a b)")


def _build_program():
    if "nc" in _CACHE:
        return _CACHE["nc"]
    nc = bacc.Bacc(
        "TRN2",
        target_bir_lowering=False,
        debug=False,
        enable_asserts=False,
        num_devices=N_CORES,
    )
    xu = nc.dram_tensor("xu", [NCHUNK, 128, FREE], F32, kind="ExternalInput")
    yo = nc.dram_tensor("yo", [NCHUNK, 128, FREE], BF16, kind="ExternalOutput")

    with tile.TileContext(nc) as tc:
        with (
            tc.tile_pool(name="consts", bufs=1) as cpool,
            tc.tile_pool(name="xp", bufs=5) as xp,
            tc.tile_pool(name="wp", bufs=3) as wp,
            tc.tile_pool(name="qp", bufs=3) as qp,
            tc.tile_pool(name="op", bufs=3) as op_,
            tc.tile_pool(name="mp", bufs=4) as mp,
        ):
            bias_n = cpool.tile([128, 1], F32, tag="bias_n")
            nc.vector.memset(bias_n[:], -MAGIC)

            xts, ws = {}, {}

            def emit_load(k):
                if k >= NCHUNK or k in xts:
                    return
                xts[k] = xp.tile([128, CI, J], F32, name="xt", tag="xt")
                nc.sync.dma_start(
                    xts[k][:],
                    bass.AP(xu, k * 128 * FREE, [[FREE, 128], [1, FREE]]),
                )

            def emit_abs(k):
                # |x| -> bf16 (full 32-ci copy); the tree reduces it below.
                if k >= NCHUNK or k in ws:
                    return
                ws[k] = wp.tile([128, CI, J], BF16, name="w", tag="w")
                nc.scalar.activation(
                    ws[k][:], xts[k][:], mybir.ActivationFunctionType.Abs,
                )

            emit_load(0)
            emit_load(1)
            emit_abs(0)
            emit_load(2)
            emit_abs(1)

            for k in range(NCHUNK):
                xt, w = xts[k], ws[k]
                emit_load(k + 3)

                # maxabs tree: bf16 max levels on DVE (flat APs); the last
                # level emits fp32 maxabs directly.
                for wdt in (16, 8, 4, 2):
                    nc.vector.tensor_tensor(
                        out=_flat(w[:, 0:wdt, :]),
                        in0=_flat(w[:, 0:wdt, :]),
                        in1=_flat(w[:, wdt : 2 * wdt, :]),
                        op=mybir.AluOpType.max,
                    )
                m32 = mp.tile([128, J], F32, tag="m32")
                nc.vector.tensor_tensor(
                    out=m32[:], in0=w[:, 0, :], in1=w[:, 1, :],
                    op=mybir.AluOpType.max,
                )

                # chunk k+2's |x| goes ahead of chunk k's ScalarE affine so
                # ABS(k) is never stuck behind a -MAGIC that waits on DVE
                # (breaks the loop-carried V->A->V cycle that set the
                # pipeline cadence at ~2x the per-chunk engine work)
                emit_abs(k + 2)

                #   sc = (bits(m32) & 0x7F800000) - 2<<23
                #   rs = NOT(sc) + 0x7F000001  (= 0x7F000000 - sc)
                sc = mp.tile([128, J], F32, tag="sc")
                rs = mp.tile([128, J], F32, tag="rs")
                nc.vector.tensor_scalar(
                    out=sc[:].bitcast(I32), in0=m32[:].bitcast(I32),
                    scalar1=0x7F800000, scalar2=None,
                    op0=mybir.AluOpType.bitwise_and,
                )
                nc.vector.tensor_scalar(
                    out=sc[:].bitcast(I32), in0=sc[:].bitcast(I32),
                    scalar1=0x01000000, scalar2=None,
                    op0=mybir.AluOpType.subtract,
                )
                nc.vector.tensor_scalar(
                    out=rs[:].bitcast(I32), in0=sc[:].bitcast(I32),
                    scalar1=-1, scalar2=0x7F000000,
                    op0=mybir.AluOpType.mult, op1=mybir.AluOpType.add,
                )
                scb = mp.tile([128, J], BF16, tag="scb")
                nc.scalar.activation(
                    scb[:], sc[:], mybir.ActivationFunctionType.Copy,
                )

                # fused scale+clip+round: r = min(max(x*rs, -c), c) + MAGIC,
                # in place on xt (one custom DVE pass replaces the multiply,
                # the dual-op clip, and the ScalarE +MAGIC affine)
                nc.vector._custom_dve(
                    _BFP_OP,
                    out=xt[:], in0=xt[:],
                    in1=rs[:].unsqueeze(1).broadcast_to([128, CI, J]),
                    s0=MAGIC, s1=CLIP_C,
                )
                # q = r - MAGIC, bf16 out (small integers).  ScalarE
                # normally; for the last chunk both this and the multiply
                # run on DVE (TS 2x / bf16-TT 2x) to shorten the tail.
                qb = qp.tile([128, CI, J], BF16, tag="qb")
                last = k == NCHUNK - 1
                if last:
                    nc.vector.tensor_scalar(
                        out=_flat(qb[:]), in0=_flat(xt[:]), scalar1=MAGIC,
                        scalar2=None, op0=mybir.AluOpType.subtract,
                    )
                else:
                    nc.scalar.activation(
                        qb[:], xt[:], mybir.ActivationFunctionType.Identity,
                        bias=bias_n[:], scale=1.0,
                    )
                # out = q * scale, bf16 (bf16 TT runs 2x on DVE even with
                # the broadcast operand; GpSimd keeps it off the DVE for
                # all but the last chunk)
                ot = op_.tile([128, CI, J], BF16, tag="ot")
                mul_eng = nc.vector if last else nc.gpsimd
                mul_eng.tensor_tensor(
                    out=ot[:], in0=qb[:],
                    in1=scb[:].unsqueeze(1).broadcast_to([128, CI, J]),
                    op=mybir.AluOpType.mult,
                )
                # store via SWDGE on the GpSimd queue: it only ever waits on
                # the multiply just above, never stalling another engine.
                store_eng = nc.sync if k == NCHUNK - 1 else nc.gpsimd
                store_eng.dma_start(
                    bass.AP(yo, k * 128 * FREE, [[FREE, 128], [1, FREE]]),
                    ot[:],
                )

    nc.compile()
    _CACHE["nc"] = nc
    return nc


def _permute_in(shard):
    # shard [4, 256, 3136] f32 -> [chunk][p 128][free],
    # p = img*32 + blk*4 + hwq, free = (ci, j), hw = hwq*784 + chunk*J + j
    t = shard.reshape(BPC, 8, CI, 4, NCHUNK, J)
    t = t.transpose(4, 0, 1, 3, 2, 5)  # [chunk, img, blk, hwq, ci, j]
    return np.ascontiguousarray(t).reshape(NCHUNK, 128, FREE)


def _permute_out(y):
    # y [chunk][p 128][free] f32 -> [4, 256, 3136]
    t = y.reshape(NCHUNK, BPC, 8, 4, CI, J)
    t = t.transpose(1, 2, 4, 3, 0, 5)  # [img, blk, ci, hwq, chunk, j]
    return np.ascontiguousarray(t).reshape(BPC, C, HW)


def kernel(activations=None, mantissa=3, blk=32, **_unused):
    x = np.ascontiguousarray(np.asarray(activations), dtype=np.float32)
    assert x.shape == (B, C, H, W), x.shape
    assert int(mantissa) == 3 and int(blk) == 32, (mantissa, blk)

    nc = _build_program()
    xr = x.reshape(B, C, HW)
    in_maps = [
        {"xu": _permute_in(xr[c * BPC : (c + 1) * BPC])} for c in range(N_CORES)
    ]
    res = run_bass_kernel_spmd(nc, in_maps, list(range(N_CORES))).results
    out = np.concatenate(
        [
            _permute_out(np.asarray(res[c]["yo"]).astype(np.float32)).reshape(
                BPC, C, H, W
            )
            for c in range(N_CORES)
        ],
        axis=0,
    )
    return out


def run_traced(activations):
    """test.py helper: run with NTFF tracing, return (out, BassKernelResults)."""
    x = np.ascontiguousarray(np.asarray(activations), dtype=np.float32)
    nc = _build_program()
    xr = x.reshape(B, C, HW)
    in_maps = [
        {"xu": _permute_in(xr[c * BPC : (c + 1) * BPC])} for c in range(N_CORES)
    ]
    r = run_bass_kernel_spmd(nc, in_maps, list(range(N_CORES)), trace=True)
    out = np.concatenate(
        [
            _permute_out(np.asarray(r.results[c]["yo"]).astype(np.float32)).reshape(
                BPC, C, H, W
            )
            for c in range(N_CORES)
        ],
        axis=0,
    )
    return out, r


# revision 18
# speedup vs baseline: 1.5208x; 1.5208x over previous
"""BFP (block floating point) activation quantization kernel for Trainium2.

Problem: x [32, 256, 56, 56] f32; per (batch, 32-channel block, h, w) the 32
channels share an exponent e = floor(log2(max |x|)); quantize each value to
q * 2^(e-2) with q = clip(round(x / 2^(e-2)), -7, 7)  (mantissa=3 bits).

Strategy (pure data-parallel over batch, 4 images per core on 8 cores):
  - The host permutes each core's shard into the exact SBUF layout
    [chunk 8][p 128][ci 32][j 98] (p = img*32 + blk*4 + hwq), so every DMA
    is a fully linear 1.6MB transfer with 12.5KB-contiguous descriptor runs
    (the in-place layout's 784B runs measured 62ns/descriptor on the SDMA
    engines, ~144 GB/s; linear runs restore ~408 GB/s).  The output is
    stored bf16 in the same permuted layout and un-permuted/upconverted on
    the host (quantized values carry at most 4 significant bits, so bf16
    is exact).
  - Per chunk: |x| on ScalarE (bf16 out; only the exponent of the maxabs
    survives), maxabs tree as bf16 max levels on DVE 2x (flat APs; the
    last level emits fp32 maxabs directly), exponent bit-ops on [128,98]
    int32, then ONE custom DVE op fuses scale+clip+round:
        r = min(max(x*rscale, -c), c) + MAGIC,  c = nextbefore(7.5)
    (clip-before-round at c is exactly clip-after-round at +-7),
    -MAGIC on ScalarE (bf16 out), out = q*scale in bf16 on GpSimd (DVE for
    the last chunk, which ends the kernel's critical tail).
  - Emission is software-pipelined: chunk k+1's |x| is enqueued on ScalarE
    before chunk k's -MAGIC so the ScalarE queue never blocks the next
    chunk's tree.  Loads issue on the SP HWDGE ring with 2-chunk lookahead;
    stores issue via GpSimd SWDGE right after the GpSimd multiply that
    produces them, so no engine queue ever waits on a foreign producer.
"""

import numpy as np

import concourse.bass as bass
import concourse.tile as tile
from concourse import bacc, mybir
from concourse import dve_ops as _DO
from concourse.bass_utils import run_bass_kernel_spmd
from concourse.dve_spec import C0, C1, Spec, Src0, Src1, lower, maxx, minn
from concourse.dve_uop import DveOpSpec

F32 = mybir.dt.float32
BF16 = mybir.dt.bfloat16
I32 = mybir.dt.int32

N_CORES = 8
B, C, H, W = 32, 256, 56, 56
HW = H * W            # 3136
BPC = B // N_CORES    # 4 images per core
NCHUNK = 8
J = HW // 4 // NCHUNK  # 98 hw-inner elements per chunk
CI = 32               # channels per block
FREE = CI * J         # free elems per chunk per partition
MAGIC = 12582912.0    # 1.5 * 2**23: RNE round-to-integer magic for |v| < 2**22
CLIP_C = 7.499999523162842  # nextbefore(7.5): round(clip(v)) == clip(round(v))

_CACHE = {}


def _register_bfp_op():
    """Custom DVE op: out = min(max(in0*in1, -s1), s1) + s0 (4 ALU stages).

    Fuses the rscale multiply, the +-qmax clip, and the +MAGIC round-add
    into one DVE pass.  in1 is the [128,1,J] rscale broadcast (STT shape).
    """
    name = "BFP_SCALE_CLIP_ROUND"
    for op in _DO.OPS:
        if op.name == name:
            return op
    spec = Spec(
        body=minn(maxx(Src0 * Src1, -C1), C1) + C0,
        reference=lambda in0, in1, s0, s1, imm2: (
            np.minimum(np.maximum(in0 * in1, -s1), s1) + s0
        ).astype(np.float32),
    )
    row = _DO._CUSTOM_DVE_ROW_BASE + len(_DO.OPS)
    shas = {
        ver: DveOpSpec(
            name=name, opcode=row, uops=lower(spec, ver=ver), rd1_en=True
        ).sha(ver)
        for ver in ("v3", "v4")
    }
    op = _DO.DveOp(name, spec, subdim=False, uops_sha=shas)
    _DO.OPS.append(op)
    _DO.CUSTOM_DVE_SPECS[name] = spec
    _DO._SUB_OPCODE_FOR_NAME[name] = row
    return op


_BFP_OP = _register_bfp_op()


def _flat(ap):
    return ap.rearrange("p a b -> p (a b)")


def _build_program():
    if "nc" in _CACHE:
        return _CACHE["nc"]
    nc = bacc.Bacc(
        "TRN2",
        target_bir_lowering=False,
        debug=False,
        enable_asserts=False,
        num_devices=N_CORES,
    )
    xu = nc.dram_tensor("xu", [NCHUNK, 128, FREE], F32, kind="ExternalInput")
    yo = nc.dram_tensor("yo", [NCHUNK, 128, FREE], BF16, kind="ExternalOutput")

    with tile.TileContext(nc) as tc:
        with (
            tc.tile_pool(name="consts", bufs=1) as cpool,
            tc.tile_pool(name="xp", bufs=5) as xp,
            tc.tile_pool(name="wp", bufs=3) as wp,
            tc.tile_pool(name="qp", bufs=3) as qp,
            tc.tile_pool(name="op", bufs=3) as op_,
            tc.tile_pool(name="mp", bufs=4) as mp,
        ):
            bias_n = cpool.tile([128, 1], F32, tag="bias_n")
            nc.vector.memset(bias_n[:], -MAGIC)

            xts, ws = {}, {}

            def emit_load(k):
                if k >= NCHUNK or k in xts:
                    return
                xts[k] = xp.tile([128, CI, J], F32, name="xt", tag="xt")
                nc.sync.dma_start(
                    xts[k][:],
                    bass.AP(xu, k * 128 * FREE, [[FREE, 128], [1, FREE]]),
                )

            def emit_abs(k):
                # |x| -> bf16 (full 32-ci copy); the tree reduces it below.
                if k >= NCHUNK or k in ws:
                    return
                ws[k] = wp.tile([128, CI, J], BF16, name="w", tag="w")
                nc.scalar.activation(
                    ws[k][:], xts[k][:], mybir.ActivationFunctionType.Abs,
                )

            emit_load(0)
            emit_load(1)
            emit_abs(0)
            emit_load(2)
            emit_abs(1)

            for k in range(NCHUNK):
                xt, w = xts[k], ws[k]
                emit_load(k + 3)

                # maxabs tree: bf16 max levels on DVE (flat APs); the last
                # level emits fp32 maxabs directly.
                for wdt in (16, 8, 4, 2):
                    nc.vector.tensor_tensor(
                        out=_flat(w[:, 0:wdt, :]),
                        in0=_flat(w[:, 0:wdt, :]),
                        in1=_flat(w[:, wdt : 2 * wdt, :]),
                        op=mybir.AluOpType.max,
                    )
                m32 = mp.tile([128, J], F32, tag="m32")
                nc.vector.tensor_tensor(
                    out=m32[:], in0=w[:, 0, :], in1=w[:, 1, :],
                    op=mybir.AluOpType.max,
                )

                # chunk k+2's |x| goes ahead of chunk k's ScalarE affine so
                # ABS(k) is never stuck behind a -MAGIC that waits on DVE
                # (breaks the loop-carried V->A->V cycle that set the
                # pipeline cadence at ~2x the per-chunk engine work)
                emit_abs(k + 2)

                #   sc = (bits(m32) & 0x7F800000) - 2<<23
                #   rs = NOT(sc) + 0x7F000001  (= 0x7F000000 - sc)
                sc = mp.tile([128, J], F32, tag="sc")
                rs = mp.tile([128, J], F32, tag="rs")
                nc.vector.tensor_scalar(
                    out=sc[:].bitcast(I32), in0=m32[:].bitcast(I32),
                    scalar1=0x7F800000, scalar2=None,
                    op0=mybir.AluOpType.bitwise_and,
                )
                nc.vector.tensor_scalar(
                    out=sc[:].bitcast(I32), in0=sc[:].bitcast(I32),
                    scalar1=0x01000000, scalar2=None,
                    op0=mybir.AluOpType.subtract,
                )
                nc.vector.tensor_scalar(
                    out=rs[:].bitcast(I32), in0=sc[:].bitcast(I32),
                    scalar1=-1, scalar2=0x7F000000,
                    op0=mybir.AluOpType.mult, op1=mybir.AluOpType.add,
                )
                scb = mp.tile([128, J], BF16, tag="scb")
                nc.scalar.activation(
                    scb[:], sc[:], mybir.ActivationFunctionType.Copy,
                )

                # fused scale+clip+round: r = min(max(x*rs, -c), c) + MAGIC,
                # in place on xt (one custom DVE pass replaces the multiply,
                # the dual-op clip, and the ScalarE +MAGIC affine)
                nc.vector._custom_dve(
                    _BFP_OP,
                    out=xt[:], in0=xt[:],
                    in1=rs[:].unsqueeze(1).broadcast_to([128, CI, J]),
                    s0=MAGIC, s1=CLIP_C,
                )
                # q = r - MAGIC, bf16 out (small integers).  ScalarE
                # normally; for the last chunk both this and the multiply
                # run on DVE (TS 2x / bf16-TT 2x) to shorten the tail.
                qb = qp.tile([128, CI, J], BF16, tag="qb")
                last = k == NCHUNK - 1
                if last:
                    nc.vector.tensor_scalar(
                        out=_flat(qb[:]), in0=_flat(xt[:]), scalar1=MAGIC,
                        scalar2=None, op0=mybir.AluOpType.subtract,
                    )
                else:
                    nc.scalar.activation(
                        qb[:], xt[:], mybir.ActivationFunctionType.Identity,
                        bias=bias_n[:], scale=1.0,
                    )
                # out = q * scale, bf16 on DVE: bf16 TT runs in the 2x perf
                # mode even with the broadcast operand (1.8us/chunk vs ~7us
                # on GpSimd), and keeping GpSimd's data streams off SBUF
                # avoids the shared-port contention that slowed every other
                # engine's ops by ~30%.
                ot = op_.tile([128, CI, J], BF16, tag="ot")
                nc.vector.tensor_tensor(
                    out=ot[:], in0=qb[:],
                    in1=scb[:].unsqueeze(1).broadcast_to([128, CI, J]),
                    op=mybir.AluOpType.mult,
                )
                # store via SWDGE on the otherwise-idle GpSimd queue (only
                # descriptor generation runs there); the last store goes on
                # the SP HWDGE ring for its lower completion latency.
                store_eng = nc.sync if last else nc.gpsimd
                store_eng.dma_start(
                    bass.AP(yo, k * 128 * FREE, [[FREE, 128], [1, FREE]]),
                    ot[:],
                )

    nc.compile()
    _CACHE["nc"] = nc
    return nc


def _permute_in(shard):
    # shard [4, 256, 3136] f32 -> [chunk][p 128][free],
    # p = img*32 + blk*4 + hwq, free = (ci, j), hw = hwq*784 + chunk*J + j
    t = shard.reshape(BPC, 8, CI, 4, NCHUNK, J)
    t = t.transpose(4, 0, 1, 3, 2, 5)  # [chunk, img, blk, hwq, ci, j]
    return np.ascontiguousarray(t).reshape(NCHUNK, 128, FREE)


def _permute_out(y):
    # y [chunk][p 128][free] f32 -> [4, 256, 3136]
    t = y.reshape(NCHUNK, BPC, 8, 4, CI, J)
    t = t.transpose(1, 2, 4, 3, 0, 5)  # [img, blk, ci, hwq, chunk, j]
    return np.ascontiguousarray(t).reshape(BPC, C, HW)


def kernel(activations=None, mantissa=3, blk=32, **_unused):
    x = np.ascontiguousarray(np.asarray(activations), dtype=np.float32)
    assert x.shape == (B, C, H, W), x.shape
    assert int(mantissa) == 3 and int(blk) == 32, (mantissa, blk)

    nc = _build_program()
    xr = x.reshape(B, C, HW)
    in_maps = [
        {"xu": _permute_in(xr[c * BPC : (c + 1) * BPC])} for c in range(N_CORES)
    ]
    res = run_bass_kernel_spmd(nc, in_maps, list(range(N_CORES))).results
    out = np.concatenate(
        [
            _permute_out(np.asarray(res[c]["yo"]).astype(np.float32)).reshape(
                BPC, C, H, W
            )
            for c in range(N_CORES)
        ],
        axis=0,
    )
    return out


def run_traced(activations):
    """test.py helper: run with NTFF tracing, return (out, BassKernelResults)."""
    x = np.ascontiguousarray(np.asarray(activations), dtype=np.float32)
    nc = _build_program()
    xr = x.reshape(B, C, HW)
    in_maps = [
        {"xu": _permute_in(xr[c * BPC : (c + 1) * BPC])} for c in range(N_CORES)
    ]
    r = run_bass_kernel_spmd(nc, in_maps, list(range(N_CORES)), trace=True)
    out = np.concatenate(
        [
            _permute_out(np.asarray(r.results[c]["yo"]).astype(np.float32)).reshape(
                BPC, C, H, W
            )
            for c in range(N_CORES)
        ],
        axis=0,
    )
    return out, r


# revision 20
# speedup vs baseline: 1.5467x; 1.0171x over previous
"""BFP (block floating point) activation quantization kernel for Trainium2.

Problem: x [32, 256, 56, 56] f32; per (batch, 32-channel block, h, w) the 32
channels share an exponent e = floor(log2(max |x|)); quantize each value to
q * 2^(e-2) with q = clip(round(x / 2^(e-2)), -7, 7)  (mantissa=3 bits).

Strategy (pure data-parallel over batch, 4 images per core on 8 cores):
  - The host permutes each core's shard into the exact SBUF layout
    [chunk 8][p 128][ci 32][j 98] (p = img*32 + blk*4 + hwq), so every DMA
    is a fully linear 1.6MB transfer with 12.5KB-contiguous descriptor runs
    (the in-place layout's 784B runs measured 62ns/descriptor on the SDMA
    engines, ~144 GB/s; linear runs restore ~408 GB/s).  The output is
    stored bf16 in the same permuted layout and un-permuted/upconverted on
    the host (quantized values carry at most 4 significant bits, so bf16
    is exact).
  - Per chunk: |x| on ScalarE (bf16 out; only the exponent of the maxabs
    survives), maxabs tree as bf16 max levels on DVE 2x (flat APs; the
    last level emits fp32 maxabs directly), exponent bit-ops on [128,98]
    int32, then ONE custom DVE op fuses scale+clip+round:
        r = min(max(x*rscale, -c), c) + MAGIC,  c = nextbefore(7.5)
    (clip-before-round at c is exactly clip-after-round at +-7),
    -MAGIC on ScalarE (bf16 out), out = q*scale in bf16 on GpSimd (DVE for
    the last chunk, which ends the kernel's critical tail).
  - Emission is software-pipelined: chunk k+1's |x| is enqueued on ScalarE
    before chunk k's -MAGIC so the ScalarE queue never blocks the next
    chunk's tree.  Loads issue on the SP HWDGE ring with 2-chunk lookahead;
    stores issue via GpSimd SWDGE right after the GpSimd multiply that
    produces them, so no engine queue ever waits on a foreign producer.
"""

import numpy as np

import concourse.bass as bass
import concourse.tile as tile
from concourse import bacc, mybir
from concourse import dve_ops as _DO
from concourse.bass_utils import run_bass_kernel_spmd
from concourse.dve_spec import C0, C1, Spec, Src0, Src1, lower, maxx, minn
from concourse.dve_uop import DveOpSpec

F32 = mybir.dt.float32
BF16 = mybir.dt.bfloat16
I32 = mybir.dt.int32

N_CORES = 8
B, C, H, W = 32, 256, 56, 56
HW = H * W            # 3136
BPC = B // N_CORES    # 4 images per core
NCHUNK = 8
J = HW // 4 // NCHUNK  # 98 hw-inner elements per chunk
CI = 32               # channels per block
FREE = CI * J         # free elems per chunk per partition
MAGIC = 12582912.0    # 1.5 * 2**23: RNE round-to-integer magic for |v| < 2**22
CLIP_C = 7.499999523162842  # nextbefore(7.5): round(clip(v)) == clip(round(v))

_CACHE = {}


def _register_bfp_op():
    """Custom DVE op: out = min(max(in0*in1, -s1), s1) + s0 (4 ALU stages).

    Fuses the rscale multiply, the +-qmax clip, and the +MAGIC round-add
    into one DVE pass.  in1 is the [128,1,J] rscale broadcast (STT shape).
    """
    name = "BFP_SCALE_CLIP_ROUND"
    for op in _DO.OPS:
        if op.name == name:
            return op
    spec = Spec(
        body=minn(maxx(Src0 * Src1, -C1), C1) + C0,
        reference=lambda in0, in1, s0, s1, imm2: (
            np.minimum(np.maximum(in0 * in1, -s1), s1) + s0
        ).astype(np.float32),
    )
    row = _DO._CUSTOM_DVE_ROW_BASE + len(_DO.OPS)
    shas = {
        ver: DveOpSpec(
            name=name, opcode=row, uops=lower(spec, ver=ver), rd1_en=True
        ).sha(ver)
        for ver in ("v3", "v4")
    }
    op = _DO.DveOp(name, spec, subdim=False, uops_sha=shas)
    _DO.OPS.append(op)
    _DO.CUSTOM_DVE_SPECS[name] = spec
    _DO._SUB_OPCODE_FOR_NAME[name] = row
    return op


_BFP_OP = _register_bfp_op()


def _flat(ap):
    return ap.rearrange("p a b -> p (a b)")


def _build_program():
    if "nc" in _CACHE:
        return _CACHE["nc"]
    nc = bacc.Bacc(
        "TRN2",
        target_bir_lowering=False,
        debug=False,
        enable_asserts=False,
        num_devices=N_CORES,
    )
    xu = nc.dram_tensor("xu", [NCHUNK, 128, FREE], F32, kind="ExternalInput")
    yo = nc.dram_tensor("yo", [NCHUNK, 128, FREE], BF16, kind="ExternalOutput")

    with tile.TileContext(nc) as tc:
        with (
            tc.tile_pool(name="consts", bufs=1) as cpool,
            tc.tile_pool(name="xp", bufs=5) as xp,
            tc.tile_pool(name="wp", bufs=3) as wp,
            tc.tile_pool(name="qp", bufs=3) as qp,
            tc.tile_pool(name="op", bufs=3) as op_,
            tc.tile_pool(name="mp", bufs=4) as mp,
        ):
            bias_n = cpool.tile([128, 1], F32, tag="bias_n")
            nc.vector.memset(bias_n[:], -MAGIC)

            xts, ws = {}, {}

            def emit_load(k):
                if k >= NCHUNK or k in xts:
                    return
                xts[k] = xp.tile([128, CI, J], F32, name="xt", tag="xt")
                if k == 0:
                    # chunk 0 loads in ci-halves so its |x| can start after
                    # half the bytes land (cuts the pipeline ramp-in)
                    for h in range(2):
                        nc.sync.dma_start(
                            xts[k][:, 16 * h : 16 * (h + 1), :],
                            bass.AP(
                                xu,
                                k * 128 * FREE + h * (FREE // 2),
                                [[FREE, 128], [1, FREE // 2]],
                            ),
                        )
                else:
                    nc.sync.dma_start(
                        xts[k][:],
                        bass.AP(xu, k * 128 * FREE, [[FREE, 128], [1, FREE]]),
                    )

            def emit_abs(k):
                # |x| -> bf16 (full 32-ci copy); the tree reduces it below.
                if k >= NCHUNK or k in ws:
                    return
                ws[k] = wp.tile([128, CI, J], BF16, name="w", tag="w")
                if k == 0:
                    for h in range(2):
                        nc.scalar.activation(
                            ws[k][:, 16 * h : 16 * (h + 1), :],
                            xts[k][:, 16 * h : 16 * (h + 1), :],
                            mybir.ActivationFunctionType.Abs,
                        )
                else:
                    nc.scalar.activation(
                        ws[k][:], xts[k][:], mybir.ActivationFunctionType.Abs,
                    )

            emit_load(0)
            emit_load(1)
            emit_abs(0)
            emit_load(2)
            emit_abs(1)

            for k in range(NCHUNK):
                xt, w = xts[k], ws[k]
                emit_load(k + 3)

                # maxabs tree: bf16 max levels on DVE (flat APs); the last
                # level emits fp32 maxabs directly.
                for wdt in (16, 8, 4, 2):
                    nc.vector.tensor_tensor(
                        out=_flat(w[:, 0:wdt, :]),
                        in0=_flat(w[:, 0:wdt, :]),
                        in1=_flat(w[:, wdt : 2 * wdt, :]),
                        op=mybir.AluOpType.max,
                    )
                m32 = mp.tile([128, J], F32, tag="m32")
                nc.vector.tensor_tensor(
                    out=m32[:], in0=w[:, 0, :], in1=w[:, 1, :],
                    op=mybir.AluOpType.max,
                )

                # chunk k+2's |x| goes ahead of chunk k's ScalarE affine so
                # ABS(k) is never stuck behind a -MAGIC that waits on DVE
                # (breaks the loop-carried V->A->V cycle that set the
                # pipeline cadence at ~2x the per-chunk engine work)
                emit_abs(k + 2)

                #   sc = (bits(m32) & 0x7F800000) - 2<<23
                #   rs = NOT(sc) + 0x7F000001  (= 0x7F000000 - sc)
                sc = mp.tile([128, J], F32, tag="sc")
                rs = mp.tile([128, J], F32, tag="rs")
                nc.vector.tensor_scalar(
                    out=sc[:].bitcast(I32), in0=m32[:].bitcast(I32),
                    scalar1=0x7F800000, scalar2=None,
                    op0=mybir.AluOpType.bitwise_and,
                )
                nc.vector.tensor_scalar(
                    out=sc[:].bitcast(I32), in0=sc[:].bitcast(I32),
                    scalar1=0x01000000, scalar2=None,
                    op0=mybir.AluOpType.subtract,
                )
                nc.vector.tensor_scalar(
                    out=rs[:].bitcast(I32), in0=sc[:].bitcast(I32),
                    scalar1=-1, scalar2=0x7F000000,
                    op0=mybir.AluOpType.mult, op1=mybir.AluOpType.add,
                )
                scb = mp.tile([128, J], BF16, tag="scb")
                nc.scalar.activation(
                    scb[:], sc[:], mybir.ActivationFunctionType.Copy,
                )

                # fused scale+clip+round: r = min(max(x*rs, -c), c) + MAGIC,
                # in place on xt (one custom DVE pass replaces the multiply,
                # the dual-op clip, and the ScalarE +MAGIC affine)
                nc.vector._custom_dve(
                    _BFP_OP,
                    out=xt[:], in0=xt[:],
                    in1=rs[:].unsqueeze(1).broadcast_to([128, CI, J]),
                    s0=MAGIC, s1=CLIP_C,
                )
                # q = r - MAGIC, bf16 out (small integers).  ScalarE
                # normally; for the last chunk both this and the multiply
                # run on DVE (TS 2x / bf16-TT 2x) to shorten the tail.
                qb = qp.tile([128, CI, J], BF16, tag="qb")
                ot = op_.tile([128, CI, J], BF16, tag="ot")
                last = k == NCHUNK - 1
                if last:
                    # tail: everything on DVE in ci-halves, each half stored
                    # as soon as its multiply lands (SP HWDGE: lower latency)
                    for h in range(2):
                        cs = slice(16 * h, 16 * (h + 1))
                        nc.vector.tensor_scalar(
                            out=_flat(qb[:, cs, :]), in0=_flat(xt[:, cs, :]),
                            scalar1=MAGIC, scalar2=None,
                            op0=mybir.AluOpType.subtract,
                        )
                        nc.vector.tensor_tensor(
                            out=ot[:, cs, :], in0=qb[:, cs, :],
                            in1=scb[:].unsqueeze(1).broadcast_to([128, 16, J]),
                            op=mybir.AluOpType.mult,
                        )
                        nc.sync.dma_start(
                            bass.AP(
                                yo,
                                k * 128 * FREE + h * (FREE // 2),
                                [[FREE, 128], [1, FREE // 2]],
                            ),
                            ot[:, cs, :],
                        )
                else:
                    # q = r - MAGIC on ScalarE, bf16 out (small integers)
                    nc.scalar.activation(
                        qb[:], xt[:], mybir.ActivationFunctionType.Identity,
                        bias=bias_n[:], scale=1.0,
                    )
                    # out = q * scale, bf16 on DVE: bf16 TT runs in the 2x
                    # perf mode even with the broadcast operand (1.8us/chunk
                    # vs ~7us on GpSimd), and keeping GpSimd's data streams
                    # off SBUF avoids the shared-port contention that slowed
                    # every other engine's ops by ~30%.
                    nc.vector.tensor_tensor(
                        out=ot[:], in0=qb[:],
                        in1=scb[:].unsqueeze(1).broadcast_to([128, CI, J]),
                        op=mybir.AluOpType.mult,
                    )
                    # store via SWDGE on the otherwise-idle GpSimd queue
                    # (only descriptor generation runs there)
                    nc.gpsimd.dma_start(
                        bass.AP(yo, k * 128 * FREE, [[FREE, 128], [1, FREE]]),
                        ot[:],
                    )

    nc.compile()
    _CACHE["nc"] = nc
    return nc


def _permute_in(shard):
    # shard [4, 256, 3136] f32 -> [chunk][p 128][free],
    # p = img*32 + blk*4 + hwq, free = (ci, j), hw = hwq*784 + chunk*J + j
    t = shard.reshape(BPC, 8, CI, 4, NCHUNK, J)
    t = t.transpose(4, 0, 1, 3, 2, 5)  # [chunk, img, blk, hwq, ci, j]
    return np.ascontiguousarray(t).reshape(NCHUNK, 128, FREE)


def _permute_out(y):
    # y [chunk][p 128][free] f32 -> [4, 256, 3136]
    t = y.reshape(NCHUNK, BPC, 8, 4, CI, J)
    t = t.transpose(1, 2, 4, 3, 0, 5)  # [img, blk, ci, hwq, chunk, j]
    return np.ascontiguousarray(t).reshape(BPC, C, HW)


def kernel(activations=None, mantissa=3, blk=32, **_unused):
    x = np.ascontiguousarray(np.asarray(activations), dtype=np.float32)
    assert x.shape == (B, C, H, W), x.shape
    assert int(mantissa) == 3 and int(blk) == 32, (mantissa, blk)

    nc = _build_program()
    xr = x.reshape(B, C, HW)
    in_maps = [
        {"xu": _permute_in(xr[c * BPC : (c + 1) * BPC])} for c in range(N_CORES)
    ]
    res = run_bass_kernel_spmd(nc, in_maps, list(range(N_CORES))).results
    out = np.concatenate(
        [
            _permute_out(np.asarray(res[c]["yo"]).astype(np.float32)).reshape(
                BPC, C, H, W
            )
            for c in range(N_CORES)
        ],
        axis=0,
    )
    return out


def run_traced(activations):
    """test.py helper: run with NTFF tracing, return (out, BassKernelResults)."""
    x = np.ascontiguousarray(np.asarray(activations), dtype=np.float32)
    nc = _build_program()
    xr = x.reshape(B, C, HW)
    in_maps = [
        {"xu": _permute_in(xr[c * BPC : (c + 1) * BPC])} for c in range(N_CORES)
    ]
    r = run_bass_kernel_spmd(nc, in_maps, list(range(N_CORES)), trace=True)
    out = np.concatenate(
        [
            _permute_out(np.asarray(r.results[c]["yo"]).astype(np.float32)).reshape(
                BPC, C, H, W
            )
            for c in range(N_CORES)
        ],
        axis=0,
    )
    return out, r


# revision 22
# speedup vs baseline: 1.6260x; 1.0513x over previous
"""BFP (block floating point) activation quantization kernel for Trainium2.

Problem: x [32, 256, 56, 56] f32; per (batch, 32-channel block, h, w) the 32
channels share an exponent e = floor(log2(max |x|)); quantize each value to
q * 2^(e-2) with q = clip(round(x / 2^(e-2)), -7, 7)  (mantissa=3 bits).

Strategy (pure data-parallel over batch, 4 images per core on 8 cores):
  - The host permutes each core's shard into the exact SBUF layout
    [chunk 8][p 128][ci 32][j 98] (p = img*32 + blk*4 + hwq), so every DMA
    is a fully linear 1.6MB transfer with 12.5KB-contiguous descriptor runs
    (the in-place layout's 784B runs measured 62ns/descriptor on the SDMA
    engines, ~144 GB/s; linear runs restore ~408 GB/s).  The output is
    stored bf16 in the same permuted layout and un-permuted/upconverted on
    the host (quantized values carry at most 4 significant bits, so bf16
    is exact).
  - Per chunk: |x| on ScalarE (bf16 out; only the exponent of the maxabs
    survives), maxabs tree as bf16 max levels on DVE 2x (flat APs; the
    last level emits fp32 maxabs directly), exponent bit-ops on [128,98]
    int32, then ONE custom DVE op fuses scale+clip+round:
        r = min(max(x*rscale, -c), c) + MAGIC,  c = nextbefore(7.5)
    (clip-before-round at c is exactly clip-after-round at +-7),
    -MAGIC on ScalarE (bf16 out), out = q*scale in bf16 on GpSimd (DVE for
    the last chunk, which ends the kernel's critical tail).
  - Emission is software-pipelined: chunk k+1's |x| is enqueued on ScalarE
    before chunk k's -MAGIC so the ScalarE queue never blocks the next
    chunk's tree.  Loads issue on the SP HWDGE ring with 2-chunk lookahead;
    stores issue via GpSimd SWDGE right after the GpSimd multiply that
    produces them, so no engine queue ever waits on a foreign producer.
"""

import numpy as np

import concourse.bass as bass
import concourse.tile as tile
from concourse import bacc, mybir
from concourse import dve_ops as _DO
from concourse.bass_utils import run_bass_kernel_spmd
from concourse.dve_spec import C0, C1, Spec, Src0, Src1, lower, maxx, minn
from concourse.dve_uop import DveOpSpec

F32 = mybir.dt.float32
BF16 = mybir.dt.bfloat16
I32 = mybir.dt.int32

N_CORES = 8
B, C, H, W = 32, 256, 56, 56
HW = H * W            # 3136
BPC = B // N_CORES    # 4 images per core
NCHUNK = 8
J = HW // 4 // NCHUNK  # 98 hw-inner elements per chunk
CI = 32               # channels per block
FREE = CI * J         # free elems per chunk per partition
MAGIC = 12582912.0    # 1.5 * 2**23: RNE round-to-integer magic for |v| < 2**22
CLIP_C = 7.499999523162842  # nextbefore(7.5): round(clip(v)) == clip(round(v))

_CACHE = {}


def _register_bfp_op():
    """Custom DVE op: out = min(max(in0*in1, -s1), s1) + s0 (4 ALU stages).

    Fuses the rscale multiply, the +-qmax clip, and the +MAGIC round-add
    into one DVE pass.  in1 is the [128,1,J] rscale broadcast (STT shape).
    """
    name = "BFP_SCALE_CLIP_ROUND"
    for op in _DO.OPS:
        if op.name == name:
            return op
    spec = Spec(
        body=minn(maxx(Src0 * Src1, -C1), C1) + C0,
        reference=lambda in0, in1, s0, s1, imm2: (
            np.minimum(np.maximum(in0 * in1, -s1), s1) + s0
        ).astype(np.float32),
    )
    row = _DO._CUSTOM_DVE_ROW_BASE + len(_DO.OPS)
    shas = {
        ver: DveOpSpec(
            name=name, opcode=row, uops=lower(spec, ver=ver), rd1_en=True
        ).sha(ver)
        for ver in ("v3", "v4")
    }
    op = _DO.DveOp(name, spec, subdim=False, uops_sha=shas)
    _DO.OPS.append(op)
    _DO.CUSTOM_DVE_SPECS[name] = spec
    _DO._SUB_OPCODE_FOR_NAME[name] = row
    return op


_BFP_OP = _register_bfp_op()


def _flat(ap):
    return ap.rearrange("p a b -> p (a b)")


def _build_program():
    if "nc" in _CACHE:
        return _CACHE["nc"]
    nc = bacc.Bacc(
        "TRN2",
        target_bir_lowering=False,
        debug=False,
        enable_asserts=False,
        num_devices=N_CORES,
    )
    xu = nc.dram_tensor("xu", [NCHUNK, 128, FREE], F32, kind="ExternalInput")
    yo = nc.dram_tensor("yo", [NCHUNK, 128, FREE], BF16, kind="ExternalOutput")

    with tile.TileContext(nc) as tc:
        with (
            tc.tile_pool(name="consts", bufs=1) as cpool,
            tc.tile_pool(name="xp", bufs=5) as xp,
            tc.tile_pool(name="wp", bufs=3) as wp,
            tc.tile_pool(name="qp", bufs=3) as qp,
            tc.tile_pool(name="op", bufs=3) as op_,
            tc.tile_pool(name="mp", bufs=4) as mp,
        ):
            bias_n = cpool.tile([128, 1], F32, tag="bias_n")
            nc.vector.memset(bias_n[:], -MAGIC)

            xts, ws = {}, {}

            def emit_load(k):
                if k >= NCHUNK or k in xts:
                    return
                xts[k] = xp.tile([128, CI, J], F32, name="xt", tag="xt")
                if k == 0:
                    # chunk 0 loads in ci-halves so its |x| can start after
                    # half the bytes land (cuts the pipeline ramp-in)
                    for h in range(2):
                        nc.sync.dma_start(
                            xts[k][:, 16 * h : 16 * (h + 1), :],
                            bass.AP(
                                xu,
                                k * 128 * FREE + h * (FREE // 2),
                                [[FREE, 128], [1, FREE // 2]],
                            ),
                        )
                else:
                    nc.sync.dma_start(
                        xts[k][:],
                        bass.AP(xu, k * 128 * FREE, [[FREE, 128], [1, FREE]]),
                    )

            def emit_abs(k):
                # |x| -> bf16 (full 32-ci copy); the tree reduces it below.
                if k >= NCHUNK or k in ws:
                    return
                ws[k] = wp.tile([128, CI, J], BF16, name="w", tag="w")
                if k == 0:
                    for h in range(2):
                        nc.scalar.activation(
                            ws[k][:, 16 * h : 16 * (h + 1), :],
                            xts[k][:, 16 * h : 16 * (h + 1), :],
                            mybir.ActivationFunctionType.Abs,
                        )
                else:
                    nc.scalar.activation(
                        ws[k][:], xts[k][:], mybir.ActivationFunctionType.Abs,
                    )

            qbs, scbs = {}, {}

            def emit_mul(k):
                # out = q * scale, bf16 on DVE: bf16 TT runs in the 2x perf
                # mode even with the broadcast operand (1.8us/chunk vs ~7us
                # on GpSimd), and keeping GpSimd's data streams off SBUF
                # avoids the shared-port contention that slowed every other
                # engine's ops by ~30%.  Store via SWDGE on the otherwise-
                # idle GpSimd queue (only descriptor generation runs there).
                if k < 0 or k not in qbs:
                    return
                ot = op_.tile([128, CI, J], BF16, name="ot", tag="ot")
                nc.vector.tensor_tensor(
                    out=ot[:], in0=qbs[k][:],
                    in1=scbs[k][:].unsqueeze(1).broadcast_to([128, CI, J]),
                    op=mybir.AluOpType.mult,
                )
                nc.gpsimd.dma_start(
                    bass.AP(yo, k * 128 * FREE, [[FREE, 128], [1, FREE]]),
                    ot[:],
                )

            emit_load(0)
            emit_load(1)
            emit_abs(0)
            emit_load(2)
            emit_abs(1)

            for k in range(NCHUNK):
                xt, w = xts[k], ws[k]
                emit_load(k + 3)

                # maxabs tree: bf16 max levels on DVE (flat APs); the last
                # level emits fp32 maxabs directly.
                for wdt in (16, 8, 4, 2):
                    nc.vector.tensor_tensor(
                        out=_flat(w[:, 0:wdt, :]),
                        in0=_flat(w[:, 0:wdt, :]),
                        in1=_flat(w[:, wdt : 2 * wdt, :]),
                        op=mybir.AluOpType.max,
                    )
                m32 = mp.tile([128, J], F32, tag="m32")
                nc.vector.tensor_tensor(
                    out=m32[:], in0=w[:, 0, :], in1=w[:, 1, :],
                    op=mybir.AluOpType.max,
                )

                # chunk k+2's |x| goes ahead of chunk k's ScalarE affine so
                # ABS(k) is never stuck behind a -MAGIC that waits on DVE
                # (breaks the loop-carried V->A->V cycle that set the
                # pipeline cadence at ~2x the per-chunk engine work)
                emit_abs(k + 2)

                #   sc = (bits(m32) & 0x7F800000) - 2<<23
                #   rs = NOT(sc) + 0x7F000001  (= 0x7F000000 - sc)
                sc = mp.tile([128, J], F32, tag="sc")
                rs = mp.tile([128, J], F32, tag="rs")
                nc.vector.tensor_scalar(
                    out=sc[:].bitcast(I32), in0=m32[:].bitcast(I32),
                    scalar1=0x7F800000, scalar2=None,
                    op0=mybir.AluOpType.bitwise_and,
                )
                nc.vector.tensor_scalar(
                    out=sc[:].bitcast(I32), in0=sc[:].bitcast(I32),
                    scalar1=0x01000000, scalar2=None,
                    op0=mybir.AluOpType.subtract,
                )
                nc.vector.tensor_scalar(
                    out=rs[:].bitcast(I32), in0=sc[:].bitcast(I32),
                    scalar1=-1, scalar2=0x7F000000,
                    op0=mybir.AluOpType.mult, op1=mybir.AluOpType.add,
                )
                scb = mp.tile([128, J], BF16, tag="scb")
                nc.scalar.activation(
                    scb[:], sc[:], mybir.ActivationFunctionType.Copy,
                )

                # fused scale+clip+round: r = min(max(x*rs, -c), c) + MAGIC,
                # in place on xt (one custom DVE pass replaces the multiply,
                # the dual-op clip, and the ScalarE +MAGIC affine)
                nc.vector._custom_dve(
                    _BFP_OP,
                    out=xt[:], in0=xt[:],
                    in1=rs[:].unsqueeze(1).broadcast_to([128, CI, J]),
                    s0=MAGIC, s1=CLIP_C,
                )
                # q = r - MAGIC, bf16 out (small integers).  ScalarE
                # normally; for the last chunk both this and the multiply
                # run on DVE (TS 2x / bf16-TT 2x) to shorten the tail.
                last = k == NCHUNK - 1
                if last:
                    # tail: everything on DVE in ci-halves, each half stored
                    # as soon as its multiply lands (SP HWDGE: lower latency)
                    qb = qp.tile([128, CI, J], BF16, tag="qb")
                    ot = op_.tile([128, CI, J], BF16, tag="ot")
                    emit_mul(k - 1)
                    for h in range(2):
                        cs = slice(16 * h, 16 * (h + 1))
                        nc.vector.tensor_scalar(
                            out=_flat(qb[:, cs, :]), in0=_flat(xt[:, cs, :]),
                            scalar1=MAGIC, scalar2=None,
                            op0=mybir.AluOpType.subtract,
                        )
                        nc.vector.tensor_tensor(
                            out=ot[:, cs, :], in0=qb[:, cs, :],
                            in1=scb[:].unsqueeze(1).broadcast_to([128, 16, J]),
                            op=mybir.AluOpType.mult,
                        )
                        nc.sync.dma_start(
                            bass.AP(
                                yo,
                                k * 128 * FREE + h * (FREE // 2),
                                [[FREE, 128], [1, FREE // 2]],
                            ),
                            ot[:, cs, :],
                        )
                else:
                    # q = r - MAGIC on ScalarE, bf16 out (small integers)
                    qb = qp.tile([128, CI, J], BF16, tag="qb")
                    nc.scalar.activation(
                        qb[:], xt[:], mybir.ActivationFunctionType.Identity,
                        bias=bias_n[:], scale=1.0,
                    )
                    qbs[k], scbs[k] = qb, scb
                    # chunk k-1's multiply lands on DVE only now: its qb has
                    # been ready since ScalarE ran under this chunk's tree,
                    # so the DVE never stalls waiting on ScalarE with the
                    # data-ready tree of the next chunk queued behind it
                    emit_mul(k - 1)

    nc.compile()
    _CACHE["nc"] = nc
    return nc


def _permute_in(shard):
    # shard [4, 256, 3136] f32 -> [chunk][p 128][free],
    # p = img*32 + blk*4 + hwq, free = (ci, j), hw = hwq*784 + chunk*J + j
    t = shard.reshape(BPC, 8, CI, 4, NCHUNK, J)
    t = t.transpose(4, 0, 1, 3, 2, 5)  # [chunk, img, blk, hwq, ci, j]
    return np.ascontiguousarray(t).reshape(NCHUNK, 128, FREE)


def _permute_out(y):
    # y [chunk][p 128][free] f32 -> [4, 256, 3136]
    t = y.reshape(NCHUNK, BPC, 8, 4, CI, J)
    t = t.transpose(1, 2, 4, 3, 0, 5)  # [img, blk, ci, hwq, chunk, j]
    return np.ascontiguousarray(t).reshape(BPC, C, HW)


def kernel(activations=None, mantissa=3, blk=32, **_unused):
    x = np.ascontiguousarray(np.asarray(activations), dtype=np.float32)
    assert x.shape == (B, C, H, W), x.shape
    assert int(mantissa) == 3 and int(blk) == 32, (mantissa, blk)

    nc = _build_program()
    xr = x.reshape(B, C, HW)
    in_maps = [
        {"xu": _permute_in(xr[c * BPC : (c + 1) * BPC])} for c in range(N_CORES)
    ]
    res = run_bass_kernel_spmd(nc, in_maps, list(range(N_CORES))).results
    out = np.concatenate(
        [
            _permute_out(np.asarray(res[c]["yo"]).astype(np.float32)).reshape(
                BPC, C, H, W
            )
            for c in range(N_CORES)
        ],
        axis=0,
    )
    return out


def run_traced(activations):
    """test.py helper: run with NTFF tracing, return (out, BassKernelResults)."""
    x = np.ascontiguousarray(np.asarray(activations), dtype=np.float32)
    nc = _build_program()
    xr = x.reshape(B, C, HW)
    in_maps = [
        {"xu": _permute_in(xr[c * BPC : (c + 1) * BPC])} for c in range(N_CORES)
    ]
    r = run_bass_kernel_spmd(nc, in_maps, list(range(N_CORES)), trace=True)
    out = np.concatenate(
        [
            _permute_out(np.asarray(r.results[c]["yo"]).astype(np.float32)).reshape(
                BPC, C, H, W
            )
            for c in range(N_CORES)
        ],
        axis=0,
    )
    return out, r
